# revision 19
# baseline (speedup 1.0000x reference)
"""ActTransNet Trainium2 kernel: 8-core SPMD, routing done host-side.

Network (B=1024, T=16, INPUT_DIM=2048, DIM=1024, N_ACTIONS=64):
    p_avg = mean_t(precondition);  e_avg = mean_t(effect)
    p_embed = p_avg @ Wp.T + bp;   e_embed = e_avg @ We.T + be
    p_t[b]  = W_trans[action[b]] @ p_embed[b]
    returns (p_t[:, None, :, None], e_embed)

Sharding strategy (hardcoded): sort samples by action on the host, split the
sorted batch into 8 contiguous chunks of 128 samples.  Each core receives its
chunk of precondition/effect, the K distinct expert matrices its chunk spans
(K = max over cores, zero-masked where unused), a one-hot sample->slot mask,
and full (replicated) projection weights.  The expert matvec is computed as
sum_k (p_embed * mask_k) @ W_sel[k].T accumulated in PSUM, which keeps the
program static SPMD while each core reads only ~K/64 of W_trans.

Precision: inputs/weights are cast to bf16 on the host (DMA and TensorE run
2x faster); all matmul accumulation is f32 in PSUM, pooling accumulates f32
on VectorE, and both outputs are written f32.

Device pipeline per core (ordered so the p-side chain that feeds the
expert transform completes as early as possible; the e-side fills gaps):
  1. x_p arrives as [J, S, T] bf16, DMA'd in 2MB chunks alternating across
     both HWDGE rings; mean-pool is a contiguous innermost-axis reduce_sum
     producing per-chunk pooled tiles directly in [j, s] matmul layout
  2. proj-p contracts over j per chunk as pooled tiles arrive; bias is a
     rank-1 ones^T @ b matmul in the same PSUM accumulation group
  3. p_embed is PE-transposed to [d, s] bf16; per-slot masked copies feed
     the expert matmuls which accumulate K x 8 d-tiles into one PSUM pair,
     paced by the W_sel stream on the scalar HWDGE ring
  4. e-side (pool, proj, store) runs under the transform's DMA shadow
"""

import sys

try:  # concourse is on sys.path in the axon images; fall back to the repo
    import concourse.bass  # noqa: F401
except ImportError:  # pragma: no cover
    sys.path.insert(0, "/opt/trn_rl_repo")

import ml_dtypes
import numpy as np

import concourse.bass as bass
import concourse.mybir as mybir
import concourse.tile as tile
from concourse import bacc, bass2jax
from concourse.masks import make_identity

# Problem shape (hardcoded per contest rules)
B, T, J, D, NA = 1024, 16, 2048, 1024, 64
NC = 8            # cores
S = B // NC       # samples per core = 128
P = 128           # partitions
JT = J // P       # 16 j-tiles (input dim)
DT = D // P       # 8 d-tiles (embed dim)
NH = D // 512     # 2 psum-bank halves of the embed dim
G = 2             # j-tiles per pooling DMA chunk
NCH = JT // G     # pooling chunks per input
F32 = mybir.dt.float32
BF16 = mybir.dt.bfloat16
NPBF16 = ml_dtypes.bfloat16

_kernel_cache: dict = {}


def _build(K: int):
    """Build the SPMD Bass program for K expert slots per core."""
    nc = bacc.Bacc(None, target_bir_lowering=False, debug=False, num_devices=NC)

    xp = nc.declare_dram_parameter("xp", [J, S, T], BF16, isOutput=False)
    xe = nc.declare_dram_parameter("xe", [J, S, T], BF16, isOutput=False)
    wp = nc.declare_dram_parameter("wp", [J // 512, P, 4, D], BF16, isOutput=False)
    we = nc.declare_dram_parameter("we", [J // 512, P, 4, D], BF16, isOutput=False)
    bp = nc.declare_dram_parameter("bp", [1, D], BF16, isOutput=False)
    be = nc.declare_dram_parameter("be", [1, D], BF16, isOutput=False)
    wsel = nc.declare_dram_parameter("wsel", [K, D, D], BF16, isOutput=False)
    mask = nc.declare_dram_parameter("mask", [P, K, S], BF16, isOutput=False)
    pt = nc.declare_dram_parameter("pt", [S, D], F32, isOutput=True)
    eo = nc.declare_dram_parameter("eo", [S, D], F32, isOutput=True)

    WPC = 4           # j-tiles per projection-weight DMA piece
    NWP = JT // WPC   # 4 pieces per projection weight

    with tile.TileContext(nc) as tc:
        with (
            tc.tile_pool(name="xpoolp", bufs=4) as xpp_p,
            tc.tile_pool(name="xpoole", bufs=4) as xpp_e,
            tc.tile_pool(name="pooled", bufs=2 * NCH) as poolp,
            tc.tile_pool(name="wproj", bufs=NWP + 2) as wpp,
            tc.tile_pool(name="wselp", bufs=6) as wsp,
            tc.tile_pool(name="emb", bufs=2) as embp,
            tc.tile_pool(name="embT", bufs=1) as embTp,
            tc.tile_pool(name="small", bufs=4) as smallp,
            tc.tile_pool(name="maskp", bufs=1) as maskp,
            tc.tile_pool(name="xm", bufs=4) as xmp,
            tc.tile_pool(name="out", bufs=2) as outp,
            tc.tile_pool(name="ps", bufs=4, space="PSUM") as psp,
            tc.tile_pool(name="tps", bufs=2, space="PSUM") as tpsp,
            tc.tile_pool(name="ps2", bufs=1, space="PSUM") as ps2p,
        ):
            # --- DMA issue preamble: FIFO order per queue = priority ---
            # sync:   wp pieces / xp evens interleaved, wsel share, pt
            # scalar: xp odds, masks, we pieces, wsel share, eo
            # gpsimd: xe chunks, e-bias, wsel tail slots
            xp_t = xp.rearrange("(jg g p) s t -> jg p g s t", g=G, p=P)
            xe_t = xe.rearrange("(jg g p) s t -> jg p g s t", g=G, p=P)
            wp_t = wp  # host pre-arranged [NWP, P, WPC, D]
            we_t = we

            xts_p, xts_e, wpcs, wecs = [], [], [], []
            # x-p first, alone on both HWDGE queues: it gates everything
            for i in range(NCH):
                eng = nc.sync if i % 2 == 0 else nc.scalar
                xt = xpp_p.tile([P, G * S * T], BF16, tag="xp", name=f"xp{i}")
                eng.dma_start(xt[:], xp_t[i])
                xts_p.append((i, xt))
            for i in range(NWP):  # wp pieces split across both queues
                eng = nc.sync if i % 2 == 0 else nc.scalar
                wt = wpp.tile([P, WPC * D], BF16, tag="wpc", name=f"wp{i}")
                eng.dma_start(wt[:], wp_t[i])
                wpcs.append(wt)
            msb = maskp.tile([P, K * S], BF16, tag="mask")
            nc.scalar.dma_start(msb[:], mask[:])
            bsb_p = smallp.tile([1, D], BF16, tag="bias_p", name="bsb_p")
            nc.scalar.dma_start(bsb_p[:], bp[:])
            # SWDGE queue: xe stream, then we pieces, then the wsel tail
            for i in range(NCH):
                xt = xpp_e.tile([P, G * S * T], BF16, tag="xe", name=f"xe{i}")
                nc.gpsimd.dma_start(xt[:], xe_t[i])
                xts_e.append((i, xt))
            for i in range(NWP):
                wt = wpp.tile([P, WPC * D], BF16, tag="wpc", name=f"we{i}")
                nc.gpsimd.dma_start(wt[:], we_t[i])
                wecs.append(wt)
            bsb_e = smallp.tile([1, D], BF16, tag="bias_e", name="bsb_e")
            nc.gpsimd.dma_start(bsb_e[:], be[:])

            ident = smallp.tile([P, P], F32, tag="ident")
            make_identity(nc, ident[:])
            ones = smallp.tile([1, P], BF16, tag="ones")
            nc.gpsimd.memset(ones[:], 1.0)

            def pool_chunks(xts, pfx):
                # reduce accumulates in the 32-bit ALU regs; only the
                # write rounds to bf16, so bf16-out loses no accuracy
                tiles = {}
                for jg, xt in xts:
                    pa = poolp.tile(
                        [P, G * S], BF16, tag="pa", name=f"pa{pfx}{jg}"
                    )
                    with nc.allow_low_precision(reason="f32 ALU accum"):
                        nc.vector.reduce_sum(
                            pa[:],
                            xt[:].rearrange("p (g s t) -> p g s t", g=G, t=T),
                            axis=mybir.AxisListType.X,
                        )
                    tiles[jg] = pa
                return [tiles[i] for i in range(NCH)]

            def project(pa_tiles, wpieces, bsb, out_dtype, pfx):
                """emb[s, d] = sum_jt pa.T @ (W.T/16) + ones.T @ b."""
                emb = embp.tile([P, D], out_dtype, tag="emb", name=f"emb{pfx}")
                psh = []
                for h in range(NH):
                    psh.append(
                        psp.tile([P, 512], F32, tag="ps", name=f"ps{pfx}{h}")
                    )
                for jt in range(JT):
                    pa = pa_tiles[jt // G]
                    gofs = (jt % G) * S
                    wt = wpieces[jt // WPC]
                    cofs = (jt % WPC) * D
                    for h in range(NH):
                        nc.tensor.matmul(
                            psh[h][:],
                            pa[:, gofs : gofs + S],
                            wt[:, cofs + h * 512 : cofs + (h + 1) * 512],
                            start=(jt == 0),
                            stop=False,
                        )
                for h in range(NH):
                    nc.tensor.matmul(
                        psh[h][:],
                        ones[:],
                        bsb[:, h * 512 : (h + 1) * 512],
                        start=False,
                        stop=True,
                    )
                    nc.scalar.activation(
                        emb[:, h * 512 : (h + 1) * 512],
                        psh[h][:],
                        mybir.ActivationFunctionType.Identity,
                    )
                return emb

            # ---- p-side chain: pool, project, transpose ----
            pa_p = pool_chunks(xts_p, "p")
            emb_p = project(pa_p, wpcs, bsb_p, F32, "p")

            pembT = embTp.tile([P, DT * S], BF16, tag="pembT")
            for dt in range(DT):
                tp = tpsp.tile([P, P], F32, tag="tps", name=f"tp{dt}")
                nc.tensor.transpose(
                    tp[:], emb_p[:, dt * P : (dt + 1) * P], ident[:]
                )
                nc.scalar.activation(
                    pembT[:, dt * S : (dt + 1) * S],
                    tp[:],
                    mybir.ActivationFunctionType.Identity,
                )

            # ---- expert transform: pt[s, i] = sum_k (pembT*m_k)^T @ Wsel_k
            DG = 2  # d-tiles per wsel DMA
            pspt = ps2p.tile([P, D], F32, tag="pspt")
            for k in range(K):
                xm = xmp.tile([P, DT * S], BF16, tag="xm", name=f"xm{k}")
                for dt in range(DT):
                    nc.vector.tensor_mul(
                        xm[:, dt * S : (dt + 1) * S],
                        pembT[:, dt * S : (dt + 1) * S],
                        msb[:, k * S : (k + 1) * S],
                    )
                wsel_k = wsel[k].rearrange("(dg g p) i -> dg p g i", g=DG, p=P)
                for dg in range(DT // DG):
                    wst = wsp.tile(
                        [P, DG * D], BF16, tag="wsel", name=f"ws{k}_{dg}"
                    )
                    # last 3 k-slots ride the (later-starting) SWDGE queue
                    if k >= K - 3:
                        eng = nc.gpsimd
                    else:
                        eng = nc.sync if (k * 4 + dg) % 2 == 0 else nc.scalar
                    eng.dma_start(wst[:], wsel_k[dg])
                    for g in range(DG):
                        dt = dg * DG + g
                        first = k == 0 and dt == 0
                        last = k == K - 1 and dt == DT - 1
                        for h in range(NH):
                            nc.tensor.matmul(
                                pspt[:, h * 512 : (h + 1) * 512],
                                xm[:, dt * S : (dt + 1) * S],
                                wst[:, g * D + h * 512 : g * D + (h + 1) * 512],
                                start=first,
                                stop=last,
                            )

            # ---- e-side: pool + project under the transform's DMA shadow
            pa_e = pool_chunks(xts_e, "e")
            emb_e = project(pa_e, wecs, bsb_e, F32, "e")
            nc.scalar.dma_start(eo[:], emb_e[:])

            ptsb = outp.tile([P, D], F32, tag="pt")
            nc.vector.tensor_copy(ptsb[:], pspt[:])
            nc.sync.dma_start(pt[:], ptsb[:])

    nc.compile()
    return nc


def _prep(precondition, effect, action, Wp, bp, We, be, W_trans):
    """Host-side routing + layout prep. Returns (in_maps, perm, K)."""
    act = np.asarray(action).astype(np.int64).ravel()
    perm = np.argsort(act, kind="stable")
    act_sorted = act[perm]

    # per-core distinct actions and sample->slot segmentation
    chunk_acts = act_sorted.reshape(NC, S)
    uniq = [np.unique(ca) for ca in chunk_acts]
    K = max(len(u) for u in uniq)

    xs_p = np.asarray(precondition, dtype=np.float32)[perm]
    xs_e = np.asarray(effect, dtype=np.float32)[perm]
    # [B, T, J] -> [J, B*T] (one cache-friendly 2D transpose) = [J, B, T],
    # then bf16; per-core slices below are contiguous row-chunk copies
    xt_p = np.ascontiguousarray(xs_p.reshape(B * T, J).T).astype(NPBF16)
    xt_p = xt_p.reshape(J, B, T)
    xt_e = np.ascontiguousarray(xs_e.reshape(B * T, J).T).astype(NPBF16)
    xt_e = xt_e.reshape(J, B, T)

    scale = np.float32(1.0 / T)
    # pieces of [P, WPC, D] with j = wc*WPC*P + c*P + p so each partition
    # reads one contiguous 8KB run per DMA piece
    WPC, NWP = 4, J // (4 * P)
    wp_t = (np.asarray(Wp, np.float32).T * scale).astype(NPBF16)
    wp_t = np.ascontiguousarray(
        wp_t.reshape(NWP, WPC, P, D).transpose(0, 2, 1, 3)
    )
    we_t = (np.asarray(We, np.float32).T * scale).astype(NPBF16)
    we_t = np.ascontiguousarray(
        we_t.reshape(NWP, WPC, P, D).transpose(0, 2, 1, 3)
    )
    bp_ = np.asarray(bp, np.float32).reshape(1, D).astype(NPBF16)
    be_ = np.asarray(be, np.float32).reshape(1, D).astype(NPBF16)
    # rhs convention needs W^T ([j, i]); transpose once globally, then bf16
    Wt = np.ascontiguousarray(
        np.asarray(W_trans, np.float32).transpose(0, 2, 1)
    ).astype(NPBF16)

    in_maps = []
    for c in range(NC):
        ca = chunk_acts[c]
        u = uniq[c]
        sel = np.zeros((K, D, D), NPBF16)
        sel[: len(u)] = Wt[u]
        slot = np.searchsorted(u, ca)  # [S] slot index per sample
        m = np.zeros((K, S), NPBF16)
        m[slot, np.arange(S)] = 1.0
        # replicate mask across the 128 j-partitions, partition-major so
        # each partition reads one contiguous K*S run: [P, K, S]
        mb = np.ascontiguousarray(np.broadcast_to(m[None, :, :], (P, K, S)))
        in_maps.append(
            {
                "xp": np.ascontiguousarray(xt_p[:, c * S : (c + 1) * S, :]),
                "xe": np.ascontiguousarray(xt_e[:, c * S : (c + 1) * S, :]),
                "wp": wp_t,
                "we": we_t,
                "bp": bp_,
                "be": be_,
                "wsel": sel,
                "mask": mb,
            }
        )
    return in_maps, perm, K


def kernel(precondition, effect, action, Wp, bp, We, be, W_trans):
    in_maps, perm, K = _prep(
        precondition, effect, action, Wp, bp, We, be, W_trans
    )
    nc = _kernel_cache.get(K)
    if nc is None:
        nc = _build(K)
        _kernel_cache[K] = nc

    results = bass2jax.run_bass_via_pjrt(nc, in_maps, n_cores=NC)

    p_sorted = np.concatenate([np.asarray(r["pt"]) for r in results], axis=0)
    e_sorted = np.concatenate([np.asarray(r["eo"]) for r in results], axis=0)
    inv = np.empty_like(perm)
    inv[perm] = np.arange(B)
    p_full = p_sorted[inv]
    e_full = e_sorted[inv]
    return (p_full[:, None, :, None].astype(np.float32),
            e_full.astype(np.float32))


# revision 20
# speedup vs baseline: 1.0925x; 1.0925x over previous
"""ActTransNet Trainium2 kernel: 8-core SPMD, routing done host-side.

Network (B=1024, T=16, INPUT_DIM=2048, DIM=1024, N_ACTIONS=64):
    p_avg = mean_t(precondition);  e_avg = mean_t(effect)
    p_embed = p_avg @ Wp.T + bp;   e_embed = e_avg @ We.T + be
    p_t[b]  = W_trans[action[b]] @ p_embed[b]
    returns (p_t[:, None, :, None], e_embed)

Sharding strategy (hardcoded): sort samples by action on the host, split the
sorted batch into 8 contiguous chunks of 128 samples.  Each core receives its
chunk of precondition/effect, the K distinct expert matrices its chunk spans
(K = max over cores, zero-masked where unused), a one-hot sample->slot mask,
and full (replicated) projection weights.  The expert matvec is computed as
sum_k (p_embed * mask_k) @ W_sel[k].T accumulated in PSUM, which keeps the
program static SPMD while each core reads only ~K/64 of W_trans.

Precision: inputs/weights are cast to bf16 on the host (DMA and TensorE run
2x faster); all matmul accumulation is f32 in PSUM, pooling accumulates f32
on VectorE, and both outputs are written f32.

Device pipeline per core (ordered so the p-side chain that feeds the
expert transform completes as early as possible; the e-side fills gaps):
  1. x_p arrives as [J, S, T] bf16, DMA'd in 2MB chunks alternating across
     both HWDGE rings; mean-pool is a contiguous innermost-axis reduce_sum
     producing per-chunk pooled tiles directly in [j, s] matmul layout
  2. proj-p contracts over j per chunk as pooled tiles arrive; bias is a
     rank-1 ones^T @ b matmul in the same PSUM accumulation group
  3. p_embed is PE-transposed to [d, s] bf16; per-slot masked copies feed
     the expert matmuls which accumulate K x 8 d-tiles into one PSUM pair,
     paced by the W_sel stream on the scalar HWDGE ring
  4. e-side (pool, proj, store) runs under the transform's DMA shadow
"""

import sys

try:  # concourse is on sys.path in the axon images; fall back to the repo
    import concourse.bass  # noqa: F401
except ImportError:  # pragma: no cover
    sys.path.insert(0, "/opt/trn_rl_repo")

import ml_dtypes
import numpy as np

import concourse.bass as bass
import concourse.mybir as mybir
import concourse.tile as tile
from concourse import bacc, bass2jax
from concourse.masks import make_identity

# Problem shape (hardcoded per contest rules)
B, T, J, D, NA = 1024, 16, 2048, 1024, 64
NC = 8            # cores
S = B // NC       # samples per core = 128
P = 128           # partitions
JT = J // P       # 16 j-tiles (input dim)
DT = D // P       # 8 d-tiles (embed dim)
NH = D // 512     # 2 psum-bank halves of the embed dim
G = 2             # j-tiles per pooling DMA chunk
NCH = JT // G     # pooling chunks per input
F32 = mybir.dt.float32
BF16 = mybir.dt.bfloat16
NPBF16 = ml_dtypes.bfloat16

_kernel_cache: dict = {}


def _build(K: int):
    """Build the SPMD Bass program for K expert slots per core."""
    nc = bacc.Bacc(None, target_bir_lowering=False, debug=False, num_devices=NC)

    xp = nc.declare_dram_parameter("xp", [J, S, T], BF16, isOutput=False)
    xe = nc.declare_dram_parameter("xe", [J, S, T], BF16, isOutput=False)
    wp = nc.declare_dram_parameter("wp", [J // 512, P, 4, D], BF16, isOutput=False)
    we = nc.declare_dram_parameter("we", [J // 512, P, 4, D], BF16, isOutput=False)
    bp = nc.declare_dram_parameter("bp", [1, D], BF16, isOutput=False)
    be = nc.declare_dram_parameter("be", [1, D], BF16, isOutput=False)
    wsel = nc.declare_dram_parameter("wsel", [K, D, D], BF16, isOutput=False)
    mask = nc.declare_dram_parameter("mask", [P, K, S], BF16, isOutput=False)
    pt = nc.declare_dram_parameter("pt", [S, D], F32, isOutput=True)
    eo = nc.declare_dram_parameter("eo", [S, D], F32, isOutput=True)

    WPC = 4           # j-tiles per projection-weight DMA piece
    NWP = JT // WPC   # 4 pieces per projection weight

    with tile.TileContext(nc) as tc:
        with (
            tc.tile_pool(name="xpoolp", bufs=4) as xpp_p,
            tc.tile_pool(name="xpoole", bufs=4) as xpp_e,
            tc.tile_pool(name="pooled", bufs=2 * NCH) as poolp,
            tc.tile_pool(name="wproj", bufs=NWP + 2) as wpp,
            tc.tile_pool(name="wselp", bufs=4) as wsp,
            tc.tile_pool(name="emb", bufs=2) as embp,
            tc.tile_pool(name="embT", bufs=1) as embTp,
            tc.tile_pool(name="small", bufs=4) as smallp,
            tc.tile_pool(name="maskp", bufs=1) as maskp,
            tc.tile_pool(name="xm", bufs=4) as xmp,
            tc.tile_pool(name="out", bufs=2) as outp,
            tc.tile_pool(name="ps", bufs=4, space="PSUM") as psp,
            tc.tile_pool(name="tps", bufs=2, space="PSUM") as tpsp,
            tc.tile_pool(name="ps2", bufs=1, space="PSUM") as ps2p,
        ):
            # --- DMA plan: two HWDGE queues only, alternated per issue so
            # each carries ~half; issue order = FIFO priority per queue.
            # x-p first (it gates the whole p-chain -> expert transform),
            # then wp pieces, then round-robin wsel / xe / we below.
            xp_t = xp.rearrange("(jg g p) s t -> jg p g s t", g=G, p=P)
            xe_t = xe.rearrange("(jg g p) s t -> jg p g s t", g=G, p=P)

            engs = [nc.sync, nc.scalar]
            ei = 0

            def eng():
                nonlocal ei
                ei += 1
                return engs[ei % 2]

            xts_p, xts_e, wpcs, wecs = [], [], [], []
            for i in range(NCH):
                xt = xpp_p.tile([P, G * S * T], BF16, tag="xp", name=f"xp{i}")
                eng().dma_start(xt[:], xp_t[i])
                xts_p.append(xt)
            for i in range(NWP):
                wt = wpp.tile([P, WPC * D], BF16, tag="wpc", name=f"wp{i}")
                eng().dma_start(wt[:], wp[i])
                wpcs.append(wt)
            msb = maskp.tile([P, K * S], BF16, tag="mask")
            eng().dma_start(msb[:], mask[:])
            bsb_p = smallp.tile([1, D], BF16, tag="bias_p", name="bsb_p")
            eng().dma_start(bsb_p[:], bp[:])
            bsb_e = smallp.tile([1, D], BF16, tag="bias_e", name="bsb_e")
            eng().dma_start(bsb_e[:], be[:])

            ident = smallp.tile([P, P], F32, tag="ident")
            make_identity(nc, ident[:])
            ones = smallp.tile([1, P], BF16, tag="ones")
            nc.gpsimd.memset(ones[:], 1.0)

            def reduce_chunk(xt, pfx, jg):
                # reduce accumulates in the 32-bit ALU regs; only the
                # write rounds to bf16, so bf16-out loses no accuracy
                pa = poolp.tile([P, G * S], BF16, tag="pa", name=f"pa{pfx}{jg}")
                with nc.allow_low_precision(reason="f32 ALU accum"):
                    nc.vector.reduce_sum(
                        pa[:],
                        xt[:].rearrange("p (g s t) -> p g s t", g=G, t=T),
                        axis=mybir.AxisListType.X,
                    )
                return pa

            def project(pa_tiles, wpieces, bsb, out_dtype, pfx):
                """emb[s, d] = sum_jt pa.T @ (W.T/16) + ones.T @ b."""
                emb = embp.tile([P, D], out_dtype, tag="emb", name=f"emb{pfx}")
                psh = []
                for h in range(NH):
                    psh.append(
                        psp.tile([P, 512], F32, tag="ps", name=f"ps{pfx}{h}")
                    )
                for jt in range(JT):
                    pa = pa_tiles[jt // G]
                    gofs = (jt % G) * S
                    wt = wpieces[jt // WPC]
                    cofs = (jt % WPC) * D
                    for h in range(NH):
                        nc.tensor.matmul(
                            psh[h][:],
                            pa[:, gofs : gofs + S],
                            wt[:, cofs + h * 512 : cofs + (h + 1) * 512],
                            start=(jt == 0),
                            stop=False,
                        )
                for h in range(NH):
                    nc.tensor.matmul(
                        psh[h][:],
                        ones[:],
                        bsb[:, h * 512 : (h + 1) * 512],
                        start=False,
                        stop=True,
                    )
                    nc.scalar.activation(
                        emb[:, h * 512 : (h + 1) * 512],
                        psh[h][:],
                        mybir.ActivationFunctionType.Identity,
                    )
                return emb

            # ---- p-side chain: pool, project, transpose ----
            pa_p = [reduce_chunk(xt, "p", i) for i, xt in enumerate(xts_p)]
            emb_p = project(pa_p, wpcs, bsb_p, F32, "p")

            pembT = embTp.tile([P, DT * S], BF16, tag="pembT")
            for dt in range(DT):
                tp = tpsp.tile([P, P], F32, tag="tps", name=f"tp{dt}")
                nc.tensor.transpose(
                    tp[:], emb_p[:, dt * P : (dt + 1) * P], ident[:]
                )
                nc.scalar.activation(
                    pembT[:, dt * S : (dt + 1) * S],
                    tp[:],
                    mybir.ActivationFunctionType.Identity,
                )

            # ---- expert transform + interleaved e-side streams ----
            # pt[s, i] = sum_k sum_dt (pembT * m_k)^T @ Wsel_k, PSUM-acc'd.
            # xe chunks / we pieces are issued round-robin between wsel
            # slots so the late e-chain never starves the wsel stream.
            DG = 4  # d-tiles per wsel DMA
            pspt = ps2p.tile([P, D], F32, tag="pspt")
            xm_tiles = {}
            for k in range(K):
                xm = xmp.tile([P, DT * S], BF16, tag="xm", name=f"xm{k}")
                for dt in range(DT):
                    nc.vector.tensor_mul(
                        xm[:, dt * S : (dt + 1) * S],
                        pembT[:, dt * S : (dt + 1) * S],
                        msb[:, k * S : (k + 1) * S],
                    )
                xm_tiles[k] = xm
                # e-side DMA interleave: one xe chunk (or we piece) per slot
                if k < NCH:
                    xt = xpp_e.tile(
                        [P, G * S * T], BF16, tag="xe", name=f"xe{k}"
                    )
                    eng().dma_start(xt[:], xe_t[k])
                    xts_e.append(xt)
                elif k - NCH < NWP:
                    wt = wpp.tile(
                        [P, WPC * D], BF16, tag="wpc", name=f"we{k - NCH}"
                    )
                    eng().dma_start(wt[:], we[k - NCH])
                    wecs.append(wt)
                wsel_k = wsel[k].rearrange("(dg g p) i -> dg p g i", g=DG, p=P)
                for dg in range(DT // DG):
                    wst = wsp.tile(
                        [P, DG * D], BF16, tag="wsel", name=f"ws{k}_{dg}"
                    )
                    eng().dma_start(wst[:], wsel_k[dg])
                    for g in range(DG):
                        dt = dg * DG + g
                        first = k == 0 and dt == 0
                        last = k == K - 1 and dt == DT - 1
                        for h in range(NH):
                            nc.tensor.matmul(
                                pspt[:, h * 512 : (h + 1) * 512],
                                xm[:, dt * S : (dt + 1) * S],
                                wst[:, g * D + h * 512 : g * D + (h + 1) * 512],
                                start=first,
                                stop=last,
                            )
                # interleave e-pooling reduce between mul/matmul groups so
                # the in-order DVE stream doesn't serialize the e-chain
                if k < NCH:
                    pass  # reduce issued next loop iter once DMA'd
            # remaining we pieces (K-NCH may be < NWP)
            for i in range(len(wecs), NWP):
                wt = wpp.tile([P, WPC * D], BF16, tag="wpc", name=f"we{i}")
                eng().dma_start(wt[:], we[i])
                wecs.append(wt)

            # ---- e-side compute: pool + project under the DMA shadow ----
            pa_e = [reduce_chunk(xt, "e", i) for i, xt in enumerate(xts_e)]
            emb_e = project(pa_e, wecs, bsb_e, F32, "e")
            eng().dma_start(eo[:], emb_e[:])

            ptsb = outp.tile([P, D], F32, tag="pt")
            nc.vector.tensor_copy(ptsb[:], pspt[:])
            eng().dma_start(pt[:], ptsb[:])

    nc.compile()
    return nc


def _prep(precondition, effect, action, Wp, bp, We, be, W_trans):
    """Host-side routing + layout prep. Returns (in_maps, perm, K)."""
    act = np.asarray(action).astype(np.int64).ravel()
    perm = np.argsort(act, kind="stable")
    act_sorted = act[perm]

    # per-core distinct actions and sample->slot segmentation
    chunk_acts = act_sorted.reshape(NC, S)
    uniq = [np.unique(ca) for ca in chunk_acts]
    K = max(len(u) for u in uniq)

    xs_p = np.asarray(precondition, dtype=np.float32)[perm]
    xs_e = np.asarray(effect, dtype=np.float32)[perm]
    # [B, T, J] -> [J, B*T] (one cache-friendly 2D transpose) = [J, B, T],
    # then bf16; per-core slices below are contiguous row-chunk copies
    xt_p = np.ascontiguousarray(xs_p.reshape(B * T, J).T).astype(NPBF16)
    xt_p = xt_p.reshape(J, B, T)
    xt_e = np.ascontiguousarray(xs_e.reshape(B * T, J).T).astype(NPBF16)
    xt_e = xt_e.reshape(J, B, T)

    scale = np.float32(1.0 / T)
    # pieces of [P, WPC, D] with j = wc*WPC*P + c*P + p so each partition
    # reads one contiguous 8KB run per DMA piece
    WPC, NWP = 4, J // (4 * P)
    wp_t = (np.asarray(Wp, np.float32).T * scale).astype(NPBF16)
    wp_t = np.ascontiguousarray(
        wp_t.reshape(NWP, WPC, P, D).transpose(0, 2, 1, 3)
    )
    we_t = (np.asarray(We, np.float32).T * scale).astype(NPBF16)
    we_t = np.ascontiguousarray(
        we_t.reshape(NWP, WPC, P, D).transpose(0, 2, 1, 3)
    )
    bp_ = np.asarray(bp, np.float32).reshape(1, D).astype(NPBF16)
    be_ = np.asarray(be, np.float32).reshape(1, D).astype(NPBF16)
    # rhs convention needs W^T ([j, i]); transpose once globally, then bf16
    Wt = np.ascontiguousarray(
        np.asarray(W_trans, np.float32).transpose(0, 2, 1)
    ).astype(NPBF16)

    in_maps = []
    for c in range(NC):
        ca = chunk_acts[c]
        u = uniq[c]
        sel = np.zeros((K, D, D), NPBF16)
        sel[: len(u)] = Wt[u]
        slot = np.searchsorted(u, ca)  # [S] slot index per sample
        m = np.zeros((K, S), NPBF16)
        m[slot, np.arange(S)] = 1.0
        # replicate mask across the 128 j-partitions, partition-major so
        # each partition reads one contiguous K*S run: [P, K, S]
        mb = np.ascontiguousarray(np.broadcast_to(m[None, :, :], (P, K, S)))
        in_maps.append(
            {
                "xp": np.ascontiguousarray(xt_p[:, c * S : (c + 1) * S, :]),
                "xe": np.ascontiguousarray(xt_e[:, c * S : (c + 1) * S, :]),
                "wp": wp_t,
                "we": we_t,
                "bp": bp_,
                "be": be_,
                "wsel": sel,
                "mask": mb,
            }
        )
    return in_maps, perm, K


def kernel(precondition, effect, action, Wp, bp, We, be, W_trans):
    in_maps, perm, K = _prep(
        precondition, effect, action, Wp, bp, We, be, W_trans
    )
    nc = _kernel_cache.get(K)
    if nc is None:
        nc = _build(K)
        _kernel_cache[K] = nc

    results = bass2jax.run_bass_via_pjrt(nc, in_maps, n_cores=NC)

    p_sorted = np.concatenate([np.asarray(r["pt"]) for r in results], axis=0)
    e_sorted = np.concatenate([np.asarray(r["eo"]) for r in results], axis=0)
    inv = np.empty_like(perm)
    inv[perm] = np.arange(B)
    p_full = p_sorted[inv]
    e_full = e_sorted[inv]
    return (p_full[:, None, :, None].astype(np.float32),
            e_full.astype(np.float32))


# revision 22
# speedup vs baseline: 1.1596x; 1.0614x over previous
"""ActTransNet Trainium2 kernel: 8-core SPMD, routing done host-side.

Network (B=1024, T=16, INPUT_DIM=2048, DIM=1024, N_ACTIONS=64):
    p_avg = mean_t(precondition);  e_avg = mean_t(effect)
    p_embed = p_avg @ Wp.T + bp;   e_embed = e_avg @ We.T + be
    p_t[b]  = W_trans[action[b]] @ p_embed[b]
    returns (p_t[:, None, :, None], e_embed)

Sharding strategy (hardcoded): sort samples by action on the host, split the
sorted batch into 8 contiguous chunks of 128 samples.  Each core receives its
chunk of precondition/effect, the K distinct expert matrices its chunk spans
(K = max over cores, zero-masked where unused), a one-hot sample->slot mask,
and full (replicated) projection weights.  The expert matvec is computed as
sum_k (p_embed * mask_k) @ W_sel[k].T accumulated in PSUM, which keeps the
program static SPMD while each core reads only ~K/64 of W_trans.

Precision: inputs/weights are cast to bf16 on the host (DMA and TensorE run
2x faster); all matmul accumulation is f32 in PSUM, pooling accumulates f32
on VectorE, and both outputs are written f32.

Device pipeline per core (ordered so the p-side chain that feeds the
expert transform completes as early as possible; the e-side fills gaps):
  1. x_p arrives as [J, S, T] bf16, DMA'd in 2MB chunks alternating across
     both HWDGE rings; mean-pool is a contiguous innermost-axis reduce_sum
     producing per-chunk pooled tiles directly in [j, s] matmul layout
  2. proj-p contracts over j per chunk as pooled tiles arrive; bias is a
     rank-1 ones^T @ b matmul in the same PSUM accumulation group
  3. p_embed is PE-transposed to [d, s] bf16; per-slot masked copies feed
     the expert matmuls which accumulate K x 8 d-tiles into one PSUM pair,
     paced by the W_sel stream on the scalar HWDGE ring
  4. e-side (pool, proj, store) runs under the transform's DMA shadow
"""

import sys

try:  # concourse is on sys.path in the axon images; fall back to the repo
    import concourse.bass  # noqa: F401
except ImportError:  # pragma: no cover
    sys.path.insert(0, "/opt/trn_rl_repo")

import ml_dtypes
import numpy as np

import concourse.bass as bass
import concourse.mybir as mybir
import concourse.tile as tile
from concourse import bacc, bass2jax
from concourse.masks import make_identity

# Problem shape (hardcoded per contest rules)
B, T, J, D, NA = 1024, 16, 2048, 1024, 64
NC = 8            # cores
S = B // NC       # samples per core = 128
P = 128           # partitions
JT = J // P       # 16 j-tiles (input dim)
DT = D // P       # 8 d-tiles (embed dim)
NH = D // 512     # 2 psum-bank halves of the embed dim
G = 2             # j-tiles per pooling DMA chunk
NCH = JT // G     # pooling chunks per input
F32 = mybir.dt.float32
BF16 = mybir.dt.bfloat16
NPBF16 = ml_dtypes.bfloat16

_kernel_cache: dict = {}


def _build(K: int):
    """Build the SPMD Bass program for K expert slots per core."""
    nc = bacc.Bacc(None, target_bir_lowering=False, debug=False, num_devices=NC)

    xp = nc.declare_dram_parameter("xp", [J, S, T], BF16, isOutput=False)
    xe = nc.declare_dram_parameter("xe", [J, S, T], BF16, isOutput=False)
    wp = nc.declare_dram_parameter("wp", [J // 512, P, 4, D], BF16, isOutput=False)
    we = nc.declare_dram_parameter("we", [J // 512, P, 4, D], BF16, isOutput=False)
    bp = nc.declare_dram_parameter("bp", [1, D], BF16, isOutput=False)
    be = nc.declare_dram_parameter("be", [1, D], BF16, isOutput=False)
    wsel = nc.declare_dram_parameter("wsel", [K, D, D], BF16, isOutput=False)
    mask = nc.declare_dram_parameter("mask", [P, K, S], BF16, isOutput=False)
    pt = nc.declare_dram_parameter("pt", [S, D], F32, isOutput=True)
    eo = nc.declare_dram_parameter("eo", [S, D], F32, isOutput=True)

    WPC = 4           # j-tiles per projection-weight DMA piece
    NWP = JT // WPC   # 4 pieces per projection weight

    with tile.TileContext(nc) as tc:
        with (
            tc.tile_pool(name="xpoolp", bufs=4) as xpp_p,
            tc.tile_pool(name="xpoole", bufs=4) as xpp_e,
            tc.tile_pool(name="pooled", bufs=2 * NCH) as poolp,
            tc.tile_pool(name="wproj", bufs=NWP + 2) as wpp,
            tc.tile_pool(name="wselp", bufs=4) as wsp,
            tc.tile_pool(name="emb", bufs=2) as embp,
            tc.tile_pool(name="embT", bufs=1) as embTp,
            tc.tile_pool(name="small", bufs=4) as smallp,
            tc.tile_pool(name="maskp", bufs=1) as maskp,
            tc.tile_pool(name="xm", bufs=4) as xmp,
            tc.tile_pool(name="out", bufs=2) as outp,
            tc.tile_pool(name="ps", bufs=4, space="PSUM") as psp,
            tc.tile_pool(name="tps", bufs=2, space="PSUM") as tpsp,
            tc.tile_pool(name="ps2", bufs=1, space="PSUM") as ps2p,
        ):
            # --- DMA plan: two HWDGE queues only, alternated per issue so
            # each carries ~half; issue order = FIFO priority per queue.
            # x-p first (it gates the whole p-chain -> expert transform),
            # then wp pieces, then round-robin wsel / xe / we below.
            xp_t = xp.rearrange("(jg g p) s t -> jg p g s t", g=G, p=P)
            xe_t = xe.rearrange("(jg g p) s t -> jg p g s t", g=G, p=P)

            engs = [nc.sync, nc.scalar]
            ei = 0

            def eng():
                nonlocal ei
                ei += 1
                return engs[ei % 2]

            xts_p, xts_e, wpcs, wecs = [], [], [], []
            for i in range(NCH):
                xt = xpp_p.tile([P, G * S * T], BF16, tag="xp", name=f"xp{i}")
                eng().dma_start(xt[:], xp_t[i])
                xts_p.append(xt)
            for i in range(NWP):
                wt = wpp.tile([P, WPC * D], BF16, tag="wpc", name=f"wp{i}")
                eng().dma_start(wt[:], wp[i])
                wpcs.append(wt)
            msb = maskp.tile([P, K * S], BF16, tag="mask")
            eng().dma_start(msb[:], mask[:])
            bsb_p = smallp.tile([1, D], BF16, tag="bias_p", name="bsb_p")
            eng().dma_start(bsb_p[:], bp[:])
            bsb_e = smallp.tile([1, D], BF16, tag="bias_e", name="bsb_e")
            eng().dma_start(bsb_e[:], be[:])

            ident = smallp.tile([P, P], F32, tag="ident")
            make_identity(nc, ident[:])
            ones = smallp.tile([1, P], BF16, tag="ones")
            nc.gpsimd.memset(ones[:], 1.0)

            def reduce_chunk(xt, pfx, jg):
                # reduce accumulates in the 32-bit ALU regs; only the
                # write rounds to bf16, so bf16-out loses no accuracy
                pa = poolp.tile([P, G * S], BF16, tag="pa", name=f"pa{pfx}{jg}")
                with nc.allow_low_precision(reason="f32 ALU accum"):
                    nc.vector.reduce_sum(
                        pa[:],
                        xt[:].rearrange("p (g s t) -> p g s t", g=G, t=T),
                        axis=mybir.AxisListType.X,
                    )
                return pa

            def project(pa_tiles, wpieces, bsb, out_dtype, pfx):
                """emb[s, d] = sum_jt pa.T @ (W.T/16) + ones.T @ b."""
                emb = embp.tile([P, D], out_dtype, tag="emb", name=f"emb{pfx}")
                psh = []
                for h in range(NH):
                    psh.append(
                        psp.tile([P, 512], F32, tag="ps", name=f"ps{pfx}{h}")
                    )
                for jt in range(JT):
                    pa = pa_tiles[jt // G]
                    gofs = (jt % G) * S
                    wt = wpieces[jt // WPC]
                    cofs = (jt % WPC) * D
                    for h in range(NH):
                        nc.tensor.matmul(
                            psh[h][:],
                            pa[:, gofs : gofs + S],
                            wt[:, cofs + h * 512 : cofs + (h + 1) * 512],
                            start=(jt == 0),
                            stop=False,
                        )
                for h in range(NH):
                    nc.tensor.matmul(
                        psh[h][:],
                        ones[:],
                        bsb[:, h * 512 : (h + 1) * 512],
                        start=False,
                        stop=True,
                    )
                    nc.scalar.activation(
                        emb[:, h * 512 : (h + 1) * 512],
                        psh[h][:],
                        mybir.ActivationFunctionType.Identity,
                    )
                return emb

            # ---- p-side chain: pool, project, transpose ----
            pa_p = [reduce_chunk(xt, "p", i) for i, xt in enumerate(xts_p)]
            emb_p = project(pa_p, wpcs, bsb_p, F32, "p")

            pembT = embTp.tile([P, DT * S], BF16, tag="pembT")
            for dt in range(DT):
                tp = tpsp.tile([P, P], F32, tag="tps", name=f"tp{dt}")
                nc.tensor.transpose(
                    tp[:], emb_p[:, dt * P : (dt + 1) * P], ident[:]
                )
                nc.scalar.activation(
                    pembT[:, dt * S : (dt + 1) * S],
                    tp[:],
                    mybir.ActivationFunctionType.Identity,
                )

            # ---- expert transform + interleaved e-side streams ----
            # pt[s, i] = sum_k sum_dt (pembT * m_k)^T @ Wsel_k, PSUM-acc'd.
            # xe chunks / we pieces are issued round-robin between wsel
            # slots so the late e-chain never starves the wsel stream.
            DG = 4  # d-tiles per wsel DMA
            pspt = ps2p.tile([P, D], F32, tag="pspt")
            xm_tiles = {}
            for k in range(K):
                xm = xmp.tile([P, DT * S], BF16, tag="xm", name=f"xm{k}")
                for dt in range(DT):
                    nc.vector.tensor_mul(
                        xm[:, dt * S : (dt + 1) * S],
                        pembT[:, dt * S : (dt + 1) * S],
                        msb[:, k * S : (k + 1) * S],
                    )
                xm_tiles[k] = xm
                # e-side DMA interleave: one xe chunk (or we piece) per slot
                if k < NCH:
                    xt = xpp_e.tile(
                        [P, G * S * T], BF16, tag="xe", name=f"xe{k}"
                    )
                    eng().dma_start(xt[:], xe_t[k])
                    xts_e.append(xt)
                elif k - NCH < NWP:
                    wt = wpp.tile(
                        [P, WPC * D], BF16, tag="wpc", name=f"we{k - NCH}"
                    )
                    eng().dma_start(wt[:], we[k - NCH])
                    wecs.append(wt)
                wsel_k = wsel[k].rearrange("(dg g p) i -> dg p g i", g=DG, p=P)
                for dg in range(DT // DG):
                    wst = wsp.tile(
                        [P, DG * D], BF16, tag="wsel", name=f"ws{k}_{dg}"
                    )
                    eng().dma_start(wst[:], wsel_k[dg])
                    for g in range(DG):
                        dt = dg * DG + g
                        first = k == 0 and dt == 0
                        last = k == K - 1 and dt == DT - 1
                        for h in range(NH):
                            nc.tensor.matmul(
                                pspt[:, h * 512 : (h + 1) * 512],
                                xm[:, dt * S : (dt + 1) * S],
                                wst[:, g * D + h * 512 : g * D + (h + 1) * 512],
                                start=first,
                                stop=last,
                            )
                # interleave e-pooling reduce between mul/matmul groups so
                # the in-order DVE stream doesn't serialize the e-chain
                if k < NCH:
                    pass  # reduce issued next loop iter once DMA'd
            # remaining we pieces (K-NCH may be < NWP)
            for i in range(len(wecs), NWP):
                wt = wpp.tile([P, WPC * D], BF16, tag="wpc", name=f"we{i}")
                eng().dma_start(wt[:], we[i])
                wecs.append(wt)

            # ---- e-side compute: pool + project under the DMA shadow ----
            pa_e = [reduce_chunk(xt, "e", i) for i, xt in enumerate(xts_e)]
            emb_e = project(pa_e, wecs, bsb_e, F32, "e")
            eng().dma_start(eo[:], emb_e[:])

            ptsb = outp.tile([P, D], F32, tag="pt")
            nc.vector.tensor_copy(ptsb[:], pspt[:])
            eng().dma_start(pt[:], ptsb[:])

    nc.compile()
    return nc


def _route(act):
    """Assign samples to cores: LPT bin-pack whole actions into 8 bins of
    exactly S samples, splitting boundary actions to fill.  Returns
    (perm [B], core_slots: per core list of (action, n_samples)) with each
    core's samples ordered slot-major."""
    cnt = np.bincount(act, minlength=NA)
    order = np.argsort(-cnt, kind="stable")
    loads = [0] * NC
    bins = [[] for _ in range(NC)]  # [(action, n)]
    for a in order:
        if cnt[a] == 0:
            continue
        i = min(range(NC), key=lambda j: loads[j])
        bins[i].append([int(a), int(cnt[a])])
        loads[i] += int(cnt[a])
    # split-fill: move excess samples from over-full to under-full bins
    for i in range(NC):
        while loads[i] > S:
            j = min(range(NC), key=lambda x: loads[x])
            take = min(loads[i] - S, S - loads[j])
            a, n = bins[i][-1]  # split the smallest (last-added) action
            move = min(take, n - 1) or take
            bins[i][-1][1] -= move
            if bins[i][-1][1] == 0:
                bins[i].pop()
            bins[j].append([a, move])
            loads[i] -= move
            loads[j] += move
    # per-action sample index queues (original order)
    sample_idx = {a: list(np.nonzero(act == a)[0]) for a in range(NA)}
    pos = {a: 0 for a in range(NA)}
    perm = np.empty(B, np.int64)
    w = 0
    core_slots = []
    for i in range(NC):
        slots = []
        for a, n in bins[i]:
            idxs = sample_idx[a][pos[a] : pos[a] + n]
            pos[a] += n
            perm[w : w + n] = idxs
            w += n
            slots.append((a, n))
        core_slots.append(slots)
    assert w == B
    return perm, core_slots


def _prep(precondition, effect, action, Wp, bp, We, be, W_trans):
    """Host-side routing + layout prep. Returns (in_maps, perm, K)."""
    act = np.asarray(action).astype(np.int64).ravel()
    perm, core_slots = _route(act)
    K = max(len(s) for s in core_slots)

    xs_p = np.asarray(precondition, dtype=np.float32)[perm]
    xs_e = np.asarray(effect, dtype=np.float32)[perm]
    # [B, T, J] -> [J, B*T] (one cache-friendly 2D transpose) = [J, B, T],
    # then bf16; per-core slices below are contiguous row-chunk copies
    xt_p = np.ascontiguousarray(xs_p.reshape(B * T, J).T).astype(NPBF16)
    xt_p = xt_p.reshape(J, B, T)
    xt_e = np.ascontiguousarray(xs_e.reshape(B * T, J).T).astype(NPBF16)
    xt_e = xt_e.reshape(J, B, T)

    scale = np.float32(1.0 / T)
    # pieces of [P, WPC, D] with j = wc*WPC*P + c*P + p so each partition
    # reads one contiguous 8KB run per DMA piece
    WPC, NWP = 4, J // (4 * P)
    wp_t = (np.asarray(Wp, np.float32).T * scale).astype(NPBF16)
    wp_t = np.ascontiguousarray(
        wp_t.reshape(NWP, WPC, P, D).transpose(0, 2, 1, 3)
    )
    we_t = (np.asarray(We, np.float32).T * scale).astype(NPBF16)
    we_t = np.ascontiguousarray(
        we_t.reshape(NWP, WPC, P, D).transpose(0, 2, 1, 3)
    )
    bp_ = np.asarray(bp, np.float32).reshape(1, D).astype(NPBF16)
    be_ = np.asarray(be, np.float32).reshape(1, D).astype(NPBF16)
    # rhs convention needs W^T ([j, i]); transpose once globally, then bf16
    Wt = np.ascontiguousarray(
        np.asarray(W_trans, np.float32).transpose(0, 2, 1)
    ).astype(NPBF16)

    in_maps = []
    for c in range(NC):
        slots = core_slots[c]
        sel = np.zeros((K, D, D), NPBF16)
        m = np.zeros((K, S), NPBF16)
        ofs = 0
        for k, (a, n) in enumerate(slots):
            sel[k] = Wt[a]
            m[k, ofs : ofs + n] = 1.0
            ofs += n
        # replicate mask across the 128 j-partitions, partition-major so
        # each partition reads one contiguous K*S run: [P, K, S]
        mb = np.ascontiguousarray(np.broadcast_to(m[None, :, :], (P, K, S)))
        in_maps.append(
            {
                "xp": np.ascontiguousarray(xt_p[:, c * S : (c + 1) * S, :]),
                "xe": np.ascontiguousarray(xt_e[:, c * S : (c + 1) * S, :]),
                "wp": wp_t,
                "we": we_t,
                "bp": bp_,
                "be": be_,
                "wsel": sel,
                "mask": mb,
            }
        )
    return in_maps, perm, K


def kernel(precondition, effect, action, Wp, bp, We, be, W_trans):
    in_maps, perm, K = _prep(
        precondition, effect, action, Wp, bp, We, be, W_trans
    )
    nc = _kernel_cache.get(K)
    if nc is None:
        nc = _build(K)
        _kernel_cache[K] = nc

    results = bass2jax.run_bass_via_pjrt(nc, in_maps, n_cores=NC)

    p_sorted = np.concatenate([np.asarray(r["pt"]) for r in results], axis=0)
    e_sorted = np.concatenate([np.asarray(r["eo"]) for r in results], axis=0)
    inv = np.empty_like(perm)
    inv[perm] = np.arange(B)
    p_full = p_sorted[inv]
    e_full = e_sorted[inv]
    return (p_full[:, None, :, None].astype(np.float32),
            e_full.astype(np.float32))


# revision 24
# speedup vs baseline: 1.2935x; 1.1155x over previous
"""ActTransNet Trainium2 kernel: 8-core SPMD, routing done host-side.

Network (B=1024, T=16, INPUT_DIM=2048, DIM=1024, N_ACTIONS=64):
    p_avg = mean_t(precondition);  e_avg = mean_t(effect)
    p_embed = p_avg @ Wp.T + bp;   e_embed = e_avg @ We.T + be
    p_t[b]  = W_trans[action[b]] @ p_embed[b]
    returns (p_t[:, None, :, None], e_embed)

Sharding strategy (hardcoded): sort samples by action on the host, split the
sorted batch into 8 contiguous chunks of 128 samples.  Each core receives its
chunk of precondition/effect, the K distinct expert matrices its chunk spans
(K = max over cores, zero-masked where unused), a one-hot sample->slot mask,
and full (replicated) projection weights.  The expert matvec is computed as
sum_k (p_embed * mask_k) @ W_sel[k].T accumulated in PSUM, which keeps the
program static SPMD while each core reads only ~K/64 of W_trans.

Precision: inputs/weights are cast to bf16 on the host (DMA and TensorE run
2x faster); all matmul accumulation is f32 in PSUM, pooling accumulates f32
on VectorE, and both outputs are written f32.

Device pipeline per core (ordered so the p-side chain that feeds the
expert transform completes as early as possible; the e-side fills gaps):
  1. x_p arrives as [J, S, T] bf16, DMA'd in 2MB chunks alternating across
     both HWDGE rings; mean-pool is a contiguous innermost-axis reduce_sum
     producing per-chunk pooled tiles directly in [j, s] matmul layout
  2. proj-p contracts over j per chunk as pooled tiles arrive; bias is a
     rank-1 ones^T @ b matmul in the same PSUM accumulation group
  3. p_embed is PE-transposed to [d, s] bf16; per-slot masked copies feed
     the expert matmuls which accumulate K x 8 d-tiles into one PSUM pair,
     paced by the W_sel stream on the scalar HWDGE ring
  4. e-side (pool, proj, store) runs under the transform's DMA shadow
"""

import sys

try:  # concourse is on sys.path in the axon images; fall back to the repo
    import concourse.bass  # noqa: F401
except ImportError:  # pragma: no cover
    sys.path.insert(0, "/opt/trn_rl_repo")

import ml_dtypes
import numpy as np

import concourse.bass as bass
import concourse.mybir as mybir
import concourse.tile as tile
from concourse import bacc, bass2jax
from concourse.masks import make_identity

# Problem shape (hardcoded per contest rules)
B, T, J, D, NA = 1024, 16, 2048, 1024, 64
NC = 8            # cores
S = B // NC       # samples per core = 128
P = 128           # partitions
JT = J // P       # 16 j-tiles (input dim)
DT = D // P       # 8 d-tiles (embed dim)
NH = D // 512     # 2 psum-bank halves of the embed dim
G = 2             # j-tiles per pooling DMA chunk
NCH = JT // G     # pooling chunks per input
F32 = mybir.dt.float32
BF16 = mybir.dt.bfloat16
NPBF16 = ml_dtypes.bfloat16

_kernel_cache: dict = {}


def _build(K: int):
    """Build the SPMD Bass program for K expert slots per core."""
    nc = bacc.Bacc(None, target_bir_lowering=False, debug=False, num_devices=NC)

    xp = nc.declare_dram_parameter("xp", [J, S, T], BF16, isOutput=False)
    xe = nc.declare_dram_parameter("xe", [J, S, T], BF16, isOutput=False)
    wp = nc.declare_dram_parameter("wp", [J // 512, P, 4, D], BF16, isOutput=False)
    we = nc.declare_dram_parameter("we", [J // 512, P, 4, D], BF16, isOutput=False)
    bp = nc.declare_dram_parameter("bp", [1, D], BF16, isOutput=False)
    be = nc.declare_dram_parameter("be", [1, D], BF16, isOutput=False)
    wsel = nc.declare_dram_parameter("wsel", [K, D, D], BF16, isOutput=False)
    mask = nc.declare_dram_parameter("mask", [P, K, S], BF16, isOutput=False)
    pt = nc.declare_dram_parameter("pt", [S, D], F32, isOutput=True)
    eo = nc.declare_dram_parameter("eo", [S, D], F32, isOutput=True)

    WPC = 4           # j-tiles per projection-weight DMA piece
    NWP = JT // WPC   # 4 pieces per projection weight

    with tile.TileContext(nc) as tc:
        with (
            tc.tile_pool(name="xpoolp", bufs=4) as xpp_p,
            tc.tile_pool(name="xpoole", bufs=4) as xpp_e,
            tc.tile_pool(name="pooled", bufs=2 * NCH) as poolp,
            tc.tile_pool(name="wproj", bufs=NWP + 2) as wpp,
            tc.tile_pool(name="wselp", bufs=4) as wsp,
            tc.tile_pool(name="emb", bufs=2) as embp,
            tc.tile_pool(name="embT", bufs=1) as embTp,
            tc.tile_pool(name="small", bufs=4) as smallp,
            tc.tile_pool(name="maskp", bufs=1) as maskp,
            tc.tile_pool(name="xm", bufs=4) as xmp,
            tc.tile_pool(name="out", bufs=2) as outp,
            tc.tile_pool(name="ps", bufs=4, space="PSUM") as psp,
            tc.tile_pool(name="tps", bufs=2, space="PSUM") as tpsp,
            tc.tile_pool(name="ps2", bufs=1, space="PSUM") as ps2p,
        ):
            # --- DMA plan: two HWDGE queues only, alternated per issue so
            # each carries ~half; issue order = FIFO priority per queue.
            # x-p first (it gates the whole p-chain -> expert transform),
            # then wp pieces, then round-robin wsel / xe / we below.
            xp_t = xp.rearrange("(jg g p) s t -> jg p g s t", g=G, p=P)
            xe_t = xe.rearrange("(jg g p) s t -> jg p g s t", g=G, p=P)

            engs = [nc.sync, nc.scalar]
            ei = 0

            def eng():
                nonlocal ei
                ei += 1
                return engs[ei % 2]

            xts_p, xts_e, wpcs, wecs = [], [], [], []
            for i in range(NCH):
                xt = xpp_p.tile([P, G * S * T], BF16, tag="xp", name=f"xp{i}")
                eng().dma_start(xt[:], xp_t[i])
                xts_p.append(xt)
            for i in range(NWP):
                wt = wpp.tile([P, WPC * D], BF16, tag="wpc", name=f"wp{i}")
                eng().dma_start(wt[:], wp[i])
                wpcs.append(wt)
            msb = maskp.tile([P, K * S], BF16, tag="mask")
            eng().dma_start(msb[:], mask[:])
            bsb_p = smallp.tile([1, D], BF16, tag="bias_p", name="bsb_p")
            eng().dma_start(bsb_p[:], bp[:])
            bsb_e = smallp.tile([1, D], BF16, tag="bias_e", name="bsb_e")
            eng().dma_start(bsb_e[:], be[:])

            ident = smallp.tile([P, P], F32, tag="ident")
            make_identity(nc, ident[:])
            ones = smallp.tile([1, P], BF16, tag="ones")
            nc.gpsimd.memset(ones[:], 1.0)

            def reduce_chunk(xt, pfx, jg):
                # reduce accumulates in the 32-bit ALU regs; only the
                # write rounds to bf16, so bf16-out loses no accuracy
                pa = poolp.tile([P, G * S], BF16, tag="pa", name=f"pa{pfx}{jg}")
                with nc.allow_low_precision(reason="f32 ALU accum"):
                    nc.vector.reduce_sum(
                        pa[:],
                        xt[:].rearrange("p (g s t) -> p g s t", g=G, t=T),
                        axis=mybir.AxisListType.X,
                    )
                return pa

            def project(pa_tiles, wpieces, bsb, out_dtype, pfx):
                """emb[s, d] = sum_jt pa.T @ (W.T/16) + ones.T @ b."""
                emb = embp.tile([P, D], out_dtype, tag="emb", name=f"emb{pfx}")
                psh = []
                for h in range(NH):
                    psh.append(
                        psp.tile([P, 512], F32, tag="ps", name=f"ps{pfx}{h}")
                    )
                for jt in range(JT):
                    pa = pa_tiles[jt // G]
                    gofs = (jt % G) * S
                    wt = wpieces[jt // WPC]
                    cofs = (jt % WPC) * D
                    for h in range(NH):
                        nc.tensor.matmul(
                            psh[h][:],
                            pa[:, gofs : gofs + S],
                            wt[:, cofs + h * 512 : cofs + (h + 1) * 512],
                            start=(jt == 0),
                            stop=False,
                        )
                for h in range(NH):
                    nc.tensor.matmul(
                        psh[h][:],
                        ones[:],
                        bsb[:, h * 512 : (h + 1) * 512],
                        start=False,
                        stop=True,
                    )
                    nc.scalar.activation(
                        emb[:, h * 512 : (h + 1) * 512],
                        psh[h][:],
                        mybir.ActivationFunctionType.Identity,
                    )
                return emb

            # ---- p-side chain: pool, project, transpose ----
            pa_p = [reduce_chunk(xt, "p", i) for i, xt in enumerate(xts_p)]
            emb_p = project(pa_p, wpcs, bsb_p, F32, "p")

            pembT = embTp.tile([P, DT * S], BF16, tag="pembT")
            for dt in range(DT):
                tp = tpsp.tile([P, P], F32, tag="tps", name=f"tp{dt}")
                nc.tensor.transpose(
                    tp[:], emb_p[:, dt * P : (dt + 1) * P], ident[:]
                )
                nc.scalar.activation(
                    pembT[:, dt * S : (dt + 1) * S],
                    tp[:],
                    mybir.ActivationFunctionType.Identity,
                )

            # ---- expert transform + interleaved e-side streams ----
            # pt[s, i] = sum_k sum_dt (pembT * m_k)^T @ Wsel_k, PSUM-acc'd.
            # xe chunks / we pieces are issued round-robin between wsel
            # slots so the late e-chain never starves the wsel stream.
            DG = 4  # d-tiles per wsel DMA
            pspt = ps2p.tile([P, D], F32, tag="pspt")
            xm_tiles = {}
            for k in range(K):
                xm = xmp.tile([P, DT * S], BF16, tag="xm", name=f"xm{k}")
                for dt in range(DT):
                    nc.vector.tensor_mul(
                        xm[:, dt * S : (dt + 1) * S],
                        pembT[:, dt * S : (dt + 1) * S],
                        msb[:, k * S : (k + 1) * S],
                    )
                xm_tiles[k] = xm
                # e-side DMA interleave, paced so every xe/we lands before
                # the final wsel tiles (whose dependent tail is shortest)
                n_eitems = NCH + NWP
                lo = (k * n_eitems) // max(K - 1, 1)
                hi = ((k + 1) * n_eitems) // max(K - 1, 1)
                for it in range(lo, min(hi, n_eitems)):
                    if it < NCH:
                        xt = xpp_e.tile(
                            [P, G * S * T], BF16, tag="xe", name=f"xe{it}"
                        )
                        eng().dma_start(xt[:], xe_t[it])
                        xts_e.append(xt)
                    else:
                        wt = wpp.tile(
                            [P, WPC * D], BF16, tag="wpc", name=f"we{it - NCH}"
                        )
                        eng().dma_start(wt[:], we[it - NCH])
                        wecs.append(wt)
                wsel_k = wsel[k].rearrange("(dg g p) i -> dg p g i", g=DG, p=P)
                for dg in range(DT // DG):
                    wst = wsp.tile(
                        [P, DG * D], BF16, tag="wsel", name=f"ws{k}_{dg}"
                    )
                    eng().dma_start(wst[:], wsel_k[dg])
                    for g in range(DG):
                        dt = dg * DG + g
                        first = k == 0 and dt == 0
                        last = k == K - 1 and dt == DT - 1
                        for h in range(NH):
                            nc.tensor.matmul(
                                pspt[:, h * 512 : (h + 1) * 512],
                                xm[:, dt * S : (dt + 1) * S],
                                wst[:, g * D + h * 512 : g * D + (h + 1) * 512],
                                start=first,
                                stop=last,
                            )
                # interleave e-pooling reduce between mul/matmul groups so
                # the in-order DVE stream doesn't serialize the e-chain
                if k < NCH:
                    pass  # reduce issued next loop iter once DMA'd
            # safety: any e-items not covered by the interleave
            for i in range(len(xts_e), NCH):
                xt = xpp_e.tile([P, G * S * T], BF16, tag="xe", name=f"xe{i}")
                eng().dma_start(xt[:], xe_t[i])
                xts_e.append(xt)
            for i in range(len(wecs), NWP):
                wt = wpp.tile([P, WPC * D], BF16, tag="wpc", name=f"we{i}")
                eng().dma_start(wt[:], we[i])
                wecs.append(wt)

            # ---- e-side compute: pool + project under the DMA shadow ----
            pa_e = [reduce_chunk(xt, "e", i) for i, xt in enumerate(xts_e)]
            emb_e = project(pa_e, wecs, bsb_e, F32, "e")
            eng().dma_start(eo[:], emb_e[:])

            ptsb = outp.tile([P, D], F32, tag="pt")
            nc.vector.tensor_copy(ptsb[:], pspt[:])
            eng().dma_start(pt[:], ptsb[:])

    nc.compile()
    return nc


def _route(act):
    """Assign samples to cores: LPT bin-pack whole actions into 8 bins of
    exactly S samples, splitting boundary actions to fill.  Returns
    (perm [B], core_slots: per core list of (action, n_samples)) with each
    core's samples ordered slot-major."""
    cnt = np.bincount(act, minlength=NA)
    order = np.argsort(-cnt, kind="stable")
    loads = [0] * NC
    bins = [[] for _ in range(NC)]  # [(action, n)]
    for a in order:
        if cnt[a] == 0:
            continue
        i = min(range(NC), key=lambda j: loads[j])
        bins[i].append([int(a), int(cnt[a])])
        loads[i] += int(cnt[a])
    # split-fill: move excess samples from over-full to under-full bins
    for i in range(NC):
        while loads[i] > S:
            j = min(range(NC), key=lambda x: loads[x])
            take = min(loads[i] - S, S - loads[j])
            a, n = bins[i][-1]  # split the smallest (last-added) action
            move = min(take, n - 1) or take
            bins[i][-1][1] -= move
            if bins[i][-1][1] == 0:
                bins[i].pop()
            bins[j].append([a, move])
            loads[i] -= move
            loads[j] += move
    # per-action sample index queues (original order)
    sample_idx = {a: list(np.nonzero(act == a)[0]) for a in range(NA)}
    pos = {a: 0 for a in range(NA)}
    perm = np.empty(B, np.int64)
    w = 0
    core_slots = []
    for i in range(NC):
        slots = []
        for a, n in bins[i]:
            idxs = sample_idx[a][pos[a] : pos[a] + n]
            pos[a] += n
            perm[w : w + n] = idxs
            w += n
            slots.append((a, n))
        core_slots.append(slots)
    assert w == B
    return perm, core_slots


def _prep(precondition, effect, action, Wp, bp, We, be, W_trans):
    """Host-side routing + layout prep. Returns (in_maps, perm, K)."""
    act = np.asarray(action).astype(np.int64).ravel()
    perm, core_slots = _route(act)
    K = max(len(s) for s in core_slots)

    xs_p = np.asarray(precondition, dtype=np.float32)[perm]
    xs_e = np.asarray(effect, dtype=np.float32)[perm]
    # [B, T, J] -> [J, B*T] (one cache-friendly 2D transpose) = [J, B, T],
    # then bf16; per-core slices below are contiguous row-chunk copies
    xt_p = np.ascontiguousarray(xs_p.reshape(B * T, J).T).astype(NPBF16)
    xt_p = xt_p.reshape(J, B, T)
    xt_e = np.ascontiguousarray(xs_e.reshape(B * T, J).T).astype(NPBF16)
    xt_e = xt_e.reshape(J, B, T)

    scale = np.float32(1.0 / T)
    # pieces of [P, WPC, D] with j = wc*WPC*P + c*P + p so each partition
    # reads one contiguous 8KB run per DMA piece
    WPC, NWP = 4, J // (4 * P)
    wp_t = (np.asarray(Wp, np.float32).T * scale).astype(NPBF16)
    wp_t = np.ascontiguousarray(
        wp_t.reshape(NWP, WPC, P, D).transpose(0, 2, 1, 3)
    )
    we_t = (np.asarray(We, np.float32).T * scale).astype(NPBF16)
    we_t = np.ascontiguousarray(
        we_t.reshape(NWP, WPC, P, D).transpose(0, 2, 1, 3)
    )
    bp_ = np.asarray(bp, np.float32).reshape(1, D).astype(NPBF16)
    be_ = np.asarray(be, np.float32).reshape(1, D).astype(NPBF16)
    # rhs convention needs W^T ([j, i]); transpose once globally, then bf16
    Wt = np.ascontiguousarray(
        np.asarray(W_trans, np.float32).transpose(0, 2, 1)
    ).astype(NPBF16)

    in_maps = []
    for c in range(NC):
        slots = core_slots[c]
        sel = np.zeros((K, D, D), NPBF16)
        m = np.zeros((K, S), NPBF16)
        ofs = 0
        for k, (a, n) in enumerate(slots):
            sel[k] = Wt[a]
            m[k, ofs : ofs + n] = 1.0
            ofs += n
        # replicate mask across the 128 j-partitions, partition-major so
        # each partition reads one contiguous K*S run: [P, K, S]
        mb = np.ascontiguousarray(np.broadcast_to(m[None, :, :], (P, K, S)))
        in_maps.append(
            {
                "xp": np.ascontiguousarray(xt_p[:, c * S : (c + 1) * S, :]),
                "xe": np.ascontiguousarray(xt_e[:, c * S : (c + 1) * S, :]),
                "wp": wp_t,
                "we": we_t,
                "bp": bp_,
                "be": be_,
                "wsel": sel,
                "mask": mb,
            }
        )
    return in_maps, perm, K


def kernel(precondition, effect, action, Wp, bp, We, be, W_trans):
    in_maps, perm, K = _prep(
        precondition, effect, action, Wp, bp, We, be, W_trans
    )
    nc = _kernel_cache.get(K)
    if nc is None:
        nc = _build(K)
        _kernel_cache[K] = nc

    results = bass2jax.run_bass_via_pjrt(nc, in_maps, n_cores=NC)

    p_sorted = np.concatenate([np.asarray(r["pt"]) for r in results], axis=0)
    e_sorted = np.concatenate([np.asarray(r["eo"]) for r in results], axis=0)
    inv = np.empty_like(perm)
    inv[perm] = np.arange(B)
    p_full = p_sorted[inv]
    e_full = e_sorted[inv]
    return (p_full[:, None, :, None].astype(np.float32),
            e_full.astype(np.float32))


# revision 26
# speedup vs baseline: 1.3386x; 1.0349x over previous
"""ActTransNet Trainium2 kernel: 8-core SPMD, routing done host-side.

Network (B=1024, T=16, INPUT_DIM=2048, DIM=1024, N_ACTIONS=64):
    p_avg = mean_t(precondition);  e_avg = mean_t(effect)
    p_embed = p_avg @ Wp.T + bp;   e_embed = e_avg @ We.T + be
    p_t[b]  = W_trans[action[b]] @ p_embed[b]
    returns (p_t[:, None, :, None], e_embed)

Sharding strategy (hardcoded): sort samples by action on the host, split the
sorted batch into 8 contiguous chunks of 128 samples.  Each core receives its
chunk of precondition/effect, the K distinct expert matrices its chunk spans
(K = max over cores, zero-masked where unused), a one-hot sample->slot mask,
and full (replicated) projection weights.  The expert matvec is computed as
sum_k (p_embed * mask_k) @ W_sel[k].T accumulated in PSUM, which keeps the
program static SPMD while each core reads only ~K/64 of W_trans.

Precision: inputs/weights are cast to bf16 on the host (DMA and TensorE run
2x faster); all matmul accumulation is f32 in PSUM, pooling accumulates f32
on VectorE, and both outputs are written f32.

Device pipeline per core (ordered so the p-side chain that feeds the
expert transform completes as early as possible; the e-side fills gaps):
  1. x_p arrives as [J, S, T] bf16, DMA'd in 2MB chunks alternating across
     both HWDGE rings; mean-pool is a contiguous innermost-axis reduce_sum
     producing per-chunk pooled tiles directly in [j, s] matmul layout
  2. proj-p contracts over j per chunk as pooled tiles arrive; bias is a
     rank-1 ones^T @ b matmul in the same PSUM accumulation group
  3. p_embed is PE-transposed to [d, s] bf16; per-slot masked copies feed
     the expert matmuls which accumulate K x 8 d-tiles into one PSUM pair,
     paced by the W_sel stream on the scalar HWDGE ring
  4. e-side (pool, proj, store) runs under the transform's DMA shadow
"""

import sys

try:  # concourse is on sys.path in the axon images; fall back to the repo
    import concourse.bass  # noqa: F401
except ImportError:  # pragma: no cover
    sys.path.insert(0, "/opt/trn_rl_repo")

import ml_dtypes
import numpy as np

import concourse.bass as bass
import concourse.mybir as mybir
import concourse.tile as tile
from concourse import bacc, bass2jax
from concourse.masks import make_identity

# Problem shape (hardcoded per contest rules)
B, T, J, D, NA = 1024, 16, 2048, 1024, 64
NC = 8            # cores
S = B // NC       # samples per core = 128
P = 128           # partitions
JT = J // P       # 16 j-tiles (input dim)
DT = D // P       # 8 d-tiles (embed dim)
NH = D // 512     # 2 psum-bank halves of the embed dim
G = 2             # j-tiles per pooling DMA chunk
NCH = JT // G     # pooling chunks per input
F32 = mybir.dt.float32
BF16 = mybir.dt.bfloat16
NPBF16 = ml_dtypes.bfloat16

_kernel_cache: dict = {}


def _build(K: int):
    """Build the SPMD Bass program for K expert slots per core."""
    nc = bacc.Bacc(None, target_bir_lowering=False, debug=False, num_devices=NC)

    xp = nc.declare_dram_parameter("xp", [J, S, T], BF16, isOutput=False)
    xe = nc.declare_dram_parameter("xe", [J, S, T], BF16, isOutput=False)
    wp = nc.declare_dram_parameter("wp", [J // 512, P, 4, D], BF16, isOutput=False)
    we = nc.declare_dram_parameter("we", [J // 512, P, 4, D], BF16, isOutput=False)
    bp = nc.declare_dram_parameter("bp", [1, D], BF16, isOutput=False)
    be = nc.declare_dram_parameter("be", [1, D], BF16, isOutput=False)
    wsel = nc.declare_dram_parameter("wsel", [K, D, D], BF16, isOutput=False)
    mask = nc.declare_dram_parameter("mask", [P, K, S], BF16, isOutput=False)
    pt = nc.declare_dram_parameter("pt", [S, D], F32, isOutput=True)
    eo = nc.declare_dram_parameter("eo", [S, D], F32, isOutput=True)

    WPC = 4           # j-tiles per projection-weight DMA piece
    NWP = JT // WPC   # 4 pieces per projection weight

    with tile.TileContext(nc) as tc:
        with (
            tc.tile_pool(name="xpoolp", bufs=4) as xpp_p,
            tc.tile_pool(name="xpoole", bufs=4) as xpp_e,
            tc.tile_pool(name="pooled", bufs=2 * NCH) as poolp,
            tc.tile_pool(name="wproj", bufs=NWP + 2) as wpp,
            tc.tile_pool(name="wselp", bufs=4) as wsp,
            tc.tile_pool(name="emb", bufs=2) as embp,
            tc.tile_pool(name="embT", bufs=1) as embTp,
            tc.tile_pool(name="small", bufs=4) as smallp,
            tc.tile_pool(name="maskp", bufs=1) as maskp,
            tc.tile_pool(name="xm", bufs=4) as xmp,
            tc.tile_pool(name="out", bufs=2) as outp,
            tc.tile_pool(name="ps", bufs=4, space="PSUM") as psp,
            tc.tile_pool(name="tps", bufs=2, space="PSUM") as tpsp,
            tc.tile_pool(name="ps2", bufs=1, space="PSUM") as ps2p,
        ):
            # --- DMA plan: two HWDGE queues only, alternated per issue so
            # each carries ~half; issue order = FIFO priority per queue.
            # x-p first (it gates the whole p-chain -> expert transform),
            # then wp pieces, then round-robin wsel / xe / we below.
            xp_t = xp.rearrange("(jg g p) s t -> jg p g s t", g=G, p=P)
            xe_t = xe.rearrange("(jg g p) s t -> jg p g s t", g=G, p=P)

            engs = [nc.sync, nc.scalar]
            ei = 0

            def eng():
                nonlocal ei
                ei += 1
                return engs[ei % 2]

            xts_p, xts_e, wpcs, wecs = [], [], [], []
            for i in range(NCH):
                xt = xpp_p.tile([P, G * S * T], BF16, tag="xp", name=f"xp{i}")
                eng().dma_start(xt[:], xp_t[i])
                xts_p.append(xt)
            for i in range(NWP):
                wt = wpp.tile([P, WPC * D], BF16, tag="wpc", name=f"wp{i}")
                eng().dma_start(wt[:], wp[i])
                wpcs.append(wt)
            msb = maskp.tile([P, K * S], BF16, tag="mask")
            eng().dma_start(msb[:], mask[:])
            bsb_p = smallp.tile([1, D], BF16, tag="bias_p", name="bsb_p")
            eng().dma_start(bsb_p[:], bp[:])
            bsb_e = smallp.tile([1, D], BF16, tag="bias_e", name="bsb_e")
            eng().dma_start(bsb_e[:], be[:])

            ident = smallp.tile([P, P], F32, tag="ident")
            make_identity(nc, ident[:])
            ones = smallp.tile([1, P], BF16, tag="ones")
            nc.gpsimd.memset(ones[:], 1.0)

            def reduce_chunk(xt, pfx, jg):
                # reduce accumulates in the 32-bit ALU regs; only the
                # write rounds to bf16, so bf16-out loses no accuracy
                pa = poolp.tile([P, G * S], BF16, tag="pa", name=f"pa{pfx}{jg}")
                with nc.allow_low_precision(reason="f32 ALU accum"):
                    nc.vector.reduce_sum(
                        pa[:],
                        xt[:].rearrange("p (g s t) -> p g s t", g=G, t=T),
                        axis=mybir.AxisListType.X,
                    )
                return pa

            def project(pa_tiles, wpieces, bsb, out_dtype, pfx):
                """emb[s, d] = sum_jt pa.T @ (W.T/16) + ones.T @ b."""
                emb = embp.tile([P, D], out_dtype, tag="emb", name=f"emb{pfx}")
                psh = []
                for h in range(NH):
                    psh.append(
                        psp.tile([P, 512], F32, tag="ps", name=f"ps{pfx}{h}")
                    )
                for jt in range(JT):
                    pa = pa_tiles[jt // G]
                    gofs = (jt % G) * S
                    wt = wpieces[jt // WPC]
                    cofs = (jt % WPC) * D
                    for h in range(NH):
                        nc.tensor.matmul(
                            psh[h][:],
                            pa[:, gofs : gofs + S],
                            wt[:, cofs + h * 512 : cofs + (h + 1) * 512],
                            start=(jt == 0),
                            stop=False,
                        )
                for h in range(NH):
                    nc.tensor.matmul(
                        psh[h][:],
                        ones[:],
                        bsb[:, h * 512 : (h + 1) * 512],
                        start=False,
                        stop=True,
                    )
                    nc.scalar.activation(
                        emb[:, h * 512 : (h + 1) * 512],
                        psh[h][:],
                        mybir.ActivationFunctionType.Identity,
                    )
                return emb

            # ---- p-side chain: pool, project, transpose ----
            pa_p = [reduce_chunk(xt, "p", i) for i, xt in enumerate(xts_p)]
            emb_p = project(pa_p, wpcs, bsb_p, F32, "p")

            pembT = embTp.tile([P, DT * S], BF16, tag="pembT")
            for dt in range(DT):
                tp = tpsp.tile([P, P], F32, tag="tps", name=f"tp{dt}")
                nc.tensor.transpose(
                    tp[:], emb_p[:, dt * P : (dt + 1) * P], ident[:]
                )
                nc.scalar.activation(
                    pembT[:, dt * S : (dt + 1) * S],
                    tp[:],
                    mybir.ActivationFunctionType.Identity,
                )

            # ---- expert transform + interleaved e-side streams ----
            # pt[s, i] = sum_k sum_dt (pembT * m_k)^T @ Wsel_k, PSUM-acc'd.
            # xe chunks / we pieces are issued round-robin between wsel
            # slots so the late e-chain never starves the wsel stream.
            DG = 4  # d-tiles per wsel DMA
            pspt = ps2p.tile([P, D], F32, tag="pspt")
            xm_tiles = {}
            for k in range(K):
                xm = xmp.tile([P, DT * S], BF16, tag="xm", name=f"xm{k}")
                for dt in range(DT):
                    nc.vector.tensor_mul(
                        xm[:, dt * S : (dt + 1) * S],
                        pembT[:, dt * S : (dt + 1) * S],
                        msb[:, k * S : (k + 1) * S],
                    )
                xm_tiles[k] = xm
                # e-side DMA interleave, paced so every xe/we lands before
                # the final wsel tiles (whose dependent tail is shortest)
                n_eitems = NCH + NWP
                lo = (k * n_eitems) // max(K - 1, 1)
                hi = ((k + 1) * n_eitems) // max(K - 1, 1)
                for it in range(lo, min(hi, n_eitems)):
                    if it < NCH:
                        xt = xpp_e.tile(
                            [P, G * S * T], BF16, tag="xe", name=f"xe{it}"
                        )
                        eng().dma_start(xt[:], xe_t[it])
                        xts_e.append(xt)
                    else:
                        wt = wpp.tile(
                            [P, WPC * D], BF16, tag="wpc", name=f"we{it - NCH}"
                        )
                        eng().dma_start(wt[:], we[it - NCH])
                        wecs.append(wt)
                wsel_k = wsel[k].rearrange("(dg g p) i -> dg p g i", g=DG, p=P)
                for dg in range(DT // DG):
                    wst = wsp.tile(
                        [P, DG * D], BF16, tag="wsel", name=f"ws{k}_{dg}"
                    )
                    eng().dma_start(wst[:], wsel_k[dg])
                    for g in range(DG):
                        dt = dg * DG + g
                        first = k == 0 and dt == 0
                        last = k == K - 1 and dt == DT - 1
                        for h in range(NH):
                            nc.tensor.matmul(
                                pspt[:, h * 512 : (h + 1) * 512],
                                xm[:, dt * S : (dt + 1) * S],
                                wst[:, g * D + h * 512 : g * D + (h + 1) * 512],
                                start=first,
                                stop=last,
                            )
                # interleave e-pooling reduce between mul/matmul groups so
                # the in-order DVE stream doesn't serialize the e-chain
                if k < NCH:
                    pass  # reduce issued next loop iter once DMA'd
            # safety: any e-items not covered by the interleave
            for i in range(len(xts_e), NCH):
                xt = xpp_e.tile([P, G * S * T], BF16, tag="xe", name=f"xe{i}")
                eng().dma_start(xt[:], xe_t[i])
                xts_e.append(xt)
            for i in range(len(wecs), NWP):
                wt = wpp.tile([P, WPC * D], BF16, tag="wpc", name=f"we{i}")
                eng().dma_start(wt[:], we[i])
                wecs.append(wt)

            # ---- e-side compute: pool + project under the DMA shadow ----
            pa_e = [reduce_chunk(xt, "e", i) for i, xt in enumerate(xts_e)]
            emb_e = project(pa_e, wecs, bsb_e, F32, "e")
            eng().dma_start(eo[:], emb_e[:])

            ptsb = outp.tile([P, D], F32, tag="pt")
            nc.vector.tensor_copy(ptsb[:], pspt[:])
            eng().dma_start(pt[:], ptsb[:])

    nc.compile()
    return nc


def _route(act):
    """Assign samples to cores: LPT bin-pack whole actions into 8 bins of
    exactly S samples, splitting boundary actions to fill.  Returns
    (perm [B], core_slots: per core list of (action, n_samples)) with each
    core's samples ordered slot-major."""
    cnt = np.bincount(act, minlength=NA)
    order = np.argsort(-cnt, kind="stable")
    loads = [0] * NC
    bins = [[] for _ in range(NC)]  # [(action, n)]
    for a in order:
        if cnt[a] == 0:
            continue
        i = min(range(NC), key=lambda j: loads[j])
        bins[i].append([int(a), int(cnt[a])])
        loads[i] += int(cnt[a])
    # split-fill: move excess samples from over-full to under-full bins
    for i in range(NC):
        while loads[i] > S:
            j = min(range(NC), key=lambda x: loads[x])
            take = min(loads[i] - S, S - loads[j])
            # split the largest action in the over-full bin
            bi = max(range(len(bins[i])), key=lambda x: bins[i][x][1])
            a, n = bins[i][bi]
            move = min(take, n)
            bins[i][bi][1] -= move
            if bins[i][bi][1] == 0:
                bins[i].pop(bi)
            bins[j].append([a, move])
            loads[i] -= move
            loads[j] += move
    # per-action sample index queues (original order)
    sample_idx = {a: list(np.nonzero(act == a)[0]) for a in range(NA)}
    pos = {a: 0 for a in range(NA)}
    perm = np.empty(B, np.int64)
    w = 0
    core_slots = []
    for i in range(NC):
        slots = []
        for a, n in bins[i]:
            idxs = sample_idx[a][pos[a] : pos[a] + n]
            pos[a] += n
            perm[w : w + n] = idxs
            w += n
            slots.append((a, n))
        core_slots.append(slots)
    assert w == B
    return perm, core_slots


def _prep(precondition, effect, action, Wp, bp, We, be, W_trans):
    """Host-side routing + layout prep. Returns (in_maps, perm, K)."""
    act = np.asarray(action).astype(np.int64).ravel()
    perm, core_slots = _route(act)
    K = max(len(s) for s in core_slots)

    xs_p = np.asarray(precondition, dtype=np.float32)[perm]
    xs_e = np.asarray(effect, dtype=np.float32)[perm]
    # [B, T, J] -> [J, B*T] (one cache-friendly 2D transpose) = [J, B, T],
    # then bf16; per-core slices below are contiguous row-chunk copies
    xt_p = np.ascontiguousarray(xs_p.reshape(B * T, J).T).astype(NPBF16)
    xt_p = xt_p.reshape(J, B, T)
    xt_e = np.ascontiguousarray(xs_e.reshape(B * T, J).T).astype(NPBF16)
    xt_e = xt_e.reshape(J, B, T)

    scale = np.float32(1.0 / T)
    # pieces of [P, WPC, D] with j = wc*WPC*P + c*P + p so each partition
    # reads one contiguous 8KB run per DMA piece
    WPC, NWP = 4, J // (4 * P)
    wp_t = (np.asarray(Wp, np.float32).T * scale).astype(NPBF16)
    wp_t = np.ascontiguousarray(
        wp_t.reshape(NWP, WPC, P, D).transpose(0, 2, 1, 3)
    )
    we_t = (np.asarray(We, np.float32).T * scale).astype(NPBF16)
    we_t = np.ascontiguousarray(
        we_t.reshape(NWP, WPC, P, D).transpose(0, 2, 1, 3)
    )
    bp_ = np.asarray(bp, np.float32).reshape(1, D).astype(NPBF16)
    be_ = np.asarray(be, np.float32).reshape(1, D).astype(NPBF16)
    # rhs convention needs W^T ([j, i]); transpose once globally, then bf16
    Wt = np.ascontiguousarray(
        np.asarray(W_trans, np.float32).transpose(0, 2, 1)
    ).astype(NPBF16)

    in_maps = []
    for c in range(NC):
        slots = core_slots[c]
        sel = np.zeros((K, D, D), NPBF16)
        m = np.zeros((K, S), NPBF16)
        ofs = 0
        for k, (a, n) in enumerate(slots):
            sel[k] = Wt[a]
            m[k, ofs : ofs + n] = 1.0
            ofs += n
        # replicate mask across the 128 j-partitions, partition-major so
        # each partition reads one contiguous K*S run: [P, K, S]
        mb = np.ascontiguousarray(np.broadcast_to(m[None, :, :], (P, K, S)))
        in_maps.append(
            {
                "xp": np.ascontiguousarray(xt_p[:, c * S : (c + 1) * S, :]),
                "xe": np.ascontiguousarray(xt_e[:, c * S : (c + 1) * S, :]),
                "wp": wp_t,
                "we": we_t,
                "bp": bp_,
                "be": be_,
                "wsel": sel,
                "mask": mb,
            }
        )
    return in_maps, perm, K


def kernel(precondition, effect, action, Wp, bp, We, be, W_trans):
    in_maps, perm, K = _prep(
        precondition, effect, action, Wp, bp, We, be, W_trans
    )
    nc = _kernel_cache.get(K)
    if nc is None:
        nc = _build(K)
        _kernel_cache[K] = nc

    results = None
    err = None
    for _ in range(3):  # transient device faults are retryable
        try:
            results = bass2jax.run_bass_via_pjrt(nc, in_maps, n_cores=NC)
            break
        except Exception as e:  # noqa: BLE001
            err = e
    if results is None:
        raise err

    p_sorted = np.concatenate([np.asarray(r["pt"]) for r in results], axis=0)
    e_sorted = np.concatenate([np.asarray(r["eo"]) for r in results], axis=0)
    inv = np.empty_like(perm)
    inv[perm] = np.arange(B)
    p_full = p_sorted[inv]
    e_full = e_sorted[inv]
    return (p_full[:, None, :, None].astype(np.float32),
            e_full.astype(np.float32))
